# revision 45
# baseline (speedup 1.0000x reference)
"""Trainium2 Bass kernel for an 8-layer transformer encoder.

B=32, S=512, D=512, H=8, F=2048, V=32000. Data-parallel over batch:
4 sequences per NeuronCore x 8 cores. All large matmuls in bfloat16
(fp32 PSUM accumulate); LayerNorm statistics in fp32/fp32r over a
bf16 residual stream. Activations stay SBUF-resident across all 8
layers (no DRAM spill); only the final LN output is DMAed out.
Weights are converted to bf16 on the host, halving weight DMA.
Score matmuls (K=64) for each head pair are emitted back-to-back at
row groups 0-63/64-127 so they execute concurrently on the PE.

Attention computes scores pre-transposed (scoresT = k @ qT), exp without
max-subtraction (scores are O(1) for this model), and appends a ones
column to V so the softmax denominator falls out of the AV matmul.
Denominators for all 8 heads are batched into one [8,S] reciprocal.

LayerNorm reduces over D (partitions) with ones-vector matmuls;
rstd = exp(-0.5*ln(var+eps)). All ACT funcs (Exp/Ln/Relu) are forced
into the single 'natural_log_exp_and_others' table set to avoid
ACT_TABLE_LOAD thrash.

Per-layer work is emitted as a cross-layer software pipeline over the
4 sequences (St1=QKV+attention, St2=O+LN1, St3a=FFN, St3b=LN2): each
LN2 matmul group is emitted just before a dense st1/st3a stream and
st1 of layer l+1 fills the layer boundary, so the tensor engine never
drains long enough for the HAM clock gate to drop to 4/8.
"""
import os
import sys

sys.path.insert(0, "/opt/trn_rl_repo")

import numpy as np

import concourse.bass as bass
import concourse.tile as tile
from concourse import bacc, mybir
from concourse.bass_utils import run_bass_kernel_spmd

F32 = mybir.dt.float32
F32R = mybir.dt.float32r
BF16 = mybir.dt.bfloat16
AF = mybir.ActivationFunctionType
ALU = mybir.AluOpType

V, L, D, H, F = 32000, 8, 512, 8, 2048
B, S = 32, 512
DK = D // H          # 64
EPS = 1e-5
NCORES = 8
SQ = B // NCORES     # 4 sequences per core
NC = D // 128        # 4 chunks of 128 over D
NF = F // 128        # 16 chunks over F
NJ = S // 128        # 4 chunks of 128 over S

N_LAYERS = int(os.environ.get("BASSK_LAYERS", str(L)))

# ---- force a single ACT table set (exp+ln+relu all live in
# 'natural_log_exp_and_others'); avoids 4 table reloads per seq-layer ----
_TABLE_TARGET = "natural_log_exp_and_others"
_orig_gat = None


def _patched_gat(arch):
    tabs = _orig_gat(arch)
    if _TABLE_TARGET in tabs:
        keep = tabs[_TABLE_TARGET]
        tabs = {name: (funcs if name == _TABLE_TARGET else funcs - keep)
                for name, funcs in tabs.items()}
    return tabs


def _install_table_patch():
    global _orig_gat
    if _orig_gat is None:
        import concourse.hw_specs as hw_specs
        _orig_gat = hw_specs.get_activation_tables
        hw_specs.get_activation_tables = _patched_gat
        bacc.get_activation_tables = _patched_gat


def _emit(nc, tc, io):
    """Emit the whole per-core program into the TileContext."""
    from contextlib import ExitStack
    ctx = ExitStack()
    sb = ctx.enter_context(tc.tile_pool(name="sb", bufs=1))
    psp = ctx.enter_context(tc.tile_pool(name="psum", bufs=8, space="PSUM"))

    def ps_tile(shape):
        return psp.tile(shape, F32, tag="ps", bufs=8, name="ps")

    # ---- program-wide constants ----
    ones_f = sb.tile([128, 1], F32, tag="ones_f", name="ones_f")
    nc.vector.memset(ones_f, 1.0)
    ones_r = sb.tile([128, 1], F32R, tag="ones_r", name="ones_r")
    nc.vector.tensor_copy(out=ones_r, in_=ones_f)
    ones_row = sb.tile([1, 128], F32, tag="ones_row", name="ones_row")
    nc.vector.memset(ones_row, 1.0)
    eps_t = sb.tile([1, 1], F32, tag="eps_t", name="eps_t")
    nc.vector.memset(eps_t, EPS)
    mask_sb = []
    for j in range(NJ):
        m = sb.tile([128, SQ], F32, tag="mask", bufs=NJ, name="mask")
        nc.sync.dma_start(out=m, in_=io["maskT"][128 * j:128 * (j + 1), :])
        mask_sb.append(m)

    def vec_tile(dram, l, n_chunks, tag):
        t = sb.tile([128, n_chunks], F32, tag=tag, bufs=2, name=tag)
        nc.sync.dma_start(out=t, in_=dram[l, :].rearrange("(c p) -> p c", p=128))
        return t

    def layernorm(r_tiles, g_v, be_v, out_dt, out_tag, out_bufs, rsq=None):
        """Returns normalized output tiles in out_dt. If rsq (squares of
        r_tiles) was precomputed upstream, the DVE work is already queued
        ahead of younger stages and the sq-sum matmuls start sooner."""
        ps_sum = ps_tile([1, S])
        for k in range(NC):
            nc.tensor.matmul(ps_sum, ones_r[:, :], r_tiles[k][:, :],
                             start=(k == 0), stop=(k == NC - 1))
        if rsq is None:
            rsq = []
            for k in range(NC):
                t = sb.tile([128, S], F32R, tag="rsq", bufs=2, name="rsq")
                nc.vector.tensor_mul(out=t,
                                     in0=r_tiles[k][:, :].bitcast(F32),
                                     in1=r_tiles[k][:, :].bitcast(F32))
                rsq.append(t)
        ps_sq = ps_tile([1, S])
        for k in range(NC):
            nc.tensor.matmul(ps_sq, ones_r[:, :], rsq[k][:, :],
                             start=(k == 0), stop=(k == NC - 1))
        mean = sb.tile([1, S], F32, tag="sm", bufs=4, name="mean")
        nc.vector.tensor_scalar_mul(out=mean, in0=ps_sum, scalar1=1.0 / D)
        m2 = sb.tile([1, S], F32, tag="sm", bufs=4, name="m2")
        nc.vector.tensor_mul(out=m2, in0=mean, in1=mean)
        var = sb.tile([1, S], F32, tag="sm", bufs=4, name="var")
        nc.vector.scalar_tensor_tensor(out=var, in0=ps_sq, scalar=1.0 / D,
                                       in1=m2, op0=ALU.mult, op1=ALU.subtract)
        # rstd = exp(-0.5 * ln(var + eps))
        nc.scalar.activation(out=var, in_=var, func=AF.Ln, bias=eps_t[:, :])
        nc.scalar.activation(out=var, in_=var, func=AF.Exp, scale=-0.5)
        mrs = sb.tile([1, S], F32, tag="sm", bufs=4, name="mrs")
        nc.vector.tensor_mul(out=mrs, in0=mean, in1=var)
        rstdB = sb.tile([128, S], F32, tag="bc", bufs=4, name="rstdB")
        nc.gpsimd.partition_broadcast(rstdB, var[0:1, :])
        mrsB = sb.tile([128, S], F32, tag="bc", bufs=4, name="mrsB")
        nc.gpsimd.partition_broadcast(mrsB, mrs[0:1, :])
        outs = []
        for k in range(NC):
            u = sb.tile([128, S], F32, tag="lnt", bufs=2, name="lnu")
            nc.vector.tensor_mul(out=u, in0=r_tiles[k][:, :].bitcast(F32),
                                 in1=rstdB)
            mg = sb.tile([128, S], F32, tag="lnt", bufs=2, name="lnmg")
            nc.vector.tensor_scalar(out=mg, in0=mrsB,
                                    scalar1=g_v[:, k:k + 1],
                                    scalar2=be_v[:, k:k + 1],
                                    op0=ALU.mult, op1=ALU.subtract)
            xo = sb.tile([128, S], out_dt, tag=out_tag, bufs=out_bufs,
                         name="xo")
            nc.vector.scalar_tensor_tensor(out=xo, in0=u,
                                           scalar=g_v[:, k:k + 1], in1=mg,
                                           op0=ALU.mult, op1=ALU.subtract)
            outs.append(xo)
        return outs

    # per-layer weight/vector tiles and per-seq intermediate state
    WT = {}
    SS = [dict() for _ in range(SQ)]

    def load_layer_weights(l, part=None):
        W = WT.setdefault(l, {})
        if part in (None, "attn"):
            wq_t, wk_t, wv_t, wo_t = [], [], [], []
            # wq/wk/wv first: they gate st1; wo is not needed until st2
            for name, lst, dram in (("wq", wq_t, io["wq"]),
                                    ("wk", wk_t, io["wk"]),
                                    ("wv", wv_t, io["wv"]),
                                    ("wo", wo_t, io["wo"])):
                for k in range(NC):
                    t = sb.tile([128, D], BF16, tag=name, bufs=8, name=name)
                    nc.sync.dma_start(
                        out=t, in_=dram[l, 128 * k:128 * (k + 1), :])
                    lst.append(t)
            W.update(wq=wq_t, wk=wk_t, wv=wv_t, wo=wo_t,
                     bq=vec_tile(io["bq"], l, NC, "bq_v"),
                     bk=vec_tile(io["bk"], l, NC, "bk_v"),
                     bo=vec_tile(io["bo2"], l, NC, "bo_v"),
                     g1=vec_tile(io["g1"], l, NC, "g1_v"),
                     be1=vec_tile(io["be1"], l, NC, "be1_v"))
        if part in (None, "ffn"):
            w1_t, w2_t = [], []
            for k in range(NC):
                t = sb.tile([128, F], BF16, tag="w1", bufs=8, name="w1")
                nc.sync.dma_start(
                    out=t, in_=io["w1"][l, 128 * k:128 * (k + 1), :])
                w1_t.append(t)
            for mf in range(NF):
                t = sb.tile([128, D], BF16, tag="w2", bufs=24, name="w2")
                nc.sync.dma_start(
                    out=t, in_=io["w2"][l, 128 * mf:128 * (mf + 1), :])
                w2_t.append(t)
            W.update(w1=w1_t, w2=w2_t,
                     b2=vec_tile(io["b2"], l, NC, "b2_v"),
                     g2=vec_tile(io["g2"], l, NC, "g2_v"),
                     be2=vec_tile(io["be2"], l, NC, "be2_v"),
                     b1=vec_tile(io["b1"], l, NF, "b1_v"))

    def load_x0(s):
        """Layer-0 input: DMA bf16 x straight into the residual ring."""
        xs16 = []
        for k in range(NC):
            t16 = sb.tile([128, S], BF16, tag="x16", bufs=16, name="x16")
            nc.sync.dma_start(out=t16,
                              in_=io["x0T"][s, 128 * k:128 * (k + 1), :])
            xs16.append(t16)
        SS[s]["x16"] = xs16

    def st1(l, s):
        """QKV + attention for sequence s."""
        W = WT[l]
        st = SS[s]
        x16 = st["x16"]
        qt, kt = [], []
        for dst, w_t, b_v, tag in ((qt, W["wq"], W["bq"], "qt"),
                                   (kt, W["wk"], W["bk"], "kt")):
            for m in range(NC):
                ps = ps_tile([128, S])
                for k in range(NC):
                    nc.tensor.matmul(ps, w_t[k][:, 128 * m:128 * (m + 1)],
                                     x16[k][:, :],
                                     start=(k == 0), stop=(k == NC - 1))
                t = sb.tile([128, S], BF16, tag=tag, bufs=4, name=tag)
                nc.vector.tensor_scalar_add(out=t, in0=ps,
                                            scalar1=b_v[:, m:m + 1])
                dst.append(t)
        vx = []
        for j in range(NJ):
            ps = ps_tile([128, D])
            for k in range(NC):
                nc.tensor.matmul(ps, x16[k][:, 128 * j:128 * (j + 1)],
                                 W["wv"][k][:, :],
                                 start=(k == 0), stop=(k == NC - 1))
            t = sb.tile([128, H, DK + 1], BF16, tag="vx", bufs=4, name="vx")
            nc.vector.tensor_copy(
                out=t[:, :, 0:DK],
                in_=ps[:].rearrange("p (h d) -> p h d", h=H))
            nc.vector.tensor_copy(
                out=t[:, :, DK:DK + 1],
                in_=ones_f[:].to_broadcast([128, H, 1]))
            vx.append(t)

        cs8 = sb.tile([H, S], F32, tag="cs8", bufs=2, name="cs8")
        oT = [None] * NC
        for c in range(NC):
            # both heads of the pair sit at row groups 0-63 / 64-127:
            # their K=64 score matmuls execute concurrently on the PE
            aT = {0: [], 1: []}
            for j in range(NJ):
                for hh in range(2):
                    off = 64 * hh
                    ps_s = ps_tile([128, S])
                    nc.tensor.matmul(
                        ps_s,
                        kt[c][off:off + DK, 128 * j:128 * (j + 1)],
                        qt[c][off:off + DK, :],
                        start=True, stop=True)
                    a = sb.tile([128, S], BF16, tag="aT", bufs=10, name="aT")
                    nc.scalar.activation(out=a, in_=ps_s, func=AF.Exp,
                                         scale=DK ** -0.5,
                                         bias=mask_sb[j][:, s:s + 1])
                    aT[hh].append(a)
            for hh in range(2):
                h, off = 2 * c + hh, 64 * hh
                ps_o = ps_tile([DK + 1, S])
                for j in range(NJ):
                    nc.tensor.matmul(ps_o, vx[j][:, h, :], aT[hh][j][:, :],
                                     start=(j == 0), stop=(j == NJ - 1))
                # drain PSUM immediately (bank release must not wait on the
                # softmax-normalize chain): unnormalized oT -> SBUF, colsum
                # row -> staging tile -> cs8
                cstmp = sb.tile([1, S], F32, tag="sm", bufs=4, name="cstmp")
                nc.scalar.copy(out=cstmp, in_=ps_o[DK:DK + 1, :])
                nc.sync.dma_start(out=cs8[h:h + 1, :], in_=cstmp[:, :])
                if oT[c] is None:
                    oT[c] = sb.tile([128, S], BF16, tag="oT", bufs=7,
                                    name="oT")
                nc.scalar.copy(out=oT[c][off:off + DK, :], in_=ps_o[0:DK, :])
        nc.vector.reciprocal(out=cs8, in_=cs8)
        # bounce reciprocals through DRAM; broadcast-DMA them back across
        # partitions (engines can't read/write unaligned partition bases,
        # DMA can; DRAM sources allow partition-stride-0 broadcast reads)
        nc.sync.dma_start(out=io["csr"][s, :, :], in_=cs8[:, :])
        for c in range(NC):
            recipB = sb.tile([128, S], F32, tag="bc", bufs=4, name="recipB")
            for half in range(2):
                src = io["csr"][s, 2 * c + half, :]
                nc.sync.dma_start(
                    out=recipB[64 * half:64 * (half + 1), :],
                    in_=bass.AP(tensor=src.tensor, offset=src.offset,
                                ap=[[0, 64]] + list(src.ap)))
            # normalize in place in SBUF, one multiply per head pair
            nc.vector.tensor_mul(out=oT[c][:, :], in0=oT[c][:, :], in1=recipB)
        st["oT"] = oT

    def st2(l, s):
        """O projection + residual + LN1."""
        W = WT[l]
        st = SS[s]
        x16, oT = st["x16"], st["oT"]
        r_tiles = []
        for m in range(NC):
            ps = ps_tile([128, S])
            for k in range(NC):
                nc.tensor.matmul(ps, W["wo"][k][:, 128 * m:128 * (m + 1)],
                                 oT[k][:, :],
                                 start=(k == 0), stop=(k == NC - 1))
            r = sb.tile([128, S], F32R, tag="r", bufs=9, name="r")
            nc.vector.scalar_tensor_tensor(
                out=r, in0=ps, scalar=W["bo"][:, m:m + 1],
                in1=x16[m][:, :], op0=ALU.add, op1=ALU.add)
            r_tiles.append(r)
        st["x16"] = None
        st["oT"] = None
        st["x1b"] = layernorm(r_tiles, W["g1"], W["be1"], BF16, "x1b", 8)

    def st3a(l, s):
        """FFN + residual; LN2 deferred to st3b. FFN2 runs after FFN1 with
        hT buffered in SBUF so only ~2 PSUM banks are live at a time
        (leaving banks for the concurrently-scheduled attention)."""
        W = WT[l]
        st = SS[s]
        x1b = st["x1b"]
        ps_f2 = [ps_tile([128, S]) for _ in range(NC)]
        for mf in range(NF):
            ps1 = ps_tile([128, S])
            for k in range(NC):
                nc.tensor.matmul(ps1, W["w1"][k][:, 128 * mf:128 * (mf + 1)],
                                 x1b[k][:, :],
                                 start=(k == 0), stop=(k == NC - 1))
            hT = sb.tile([128, S], BF16, tag="hT", bufs=2, name="hT")
            nc.scalar.activation(out=hT, in_=ps1, func=AF.Relu,
                                 bias=W["b1"][:, mf:mf + 1])
            for m2 in range(NC):
                nc.tensor.matmul(ps_f2[m2],
                                 W["w2"][mf][:, 128 * m2:128 * (m2 + 1)],
                                 hT[:, :],
                                 start=(mf == 0), stop=(mf == NF - 1))
        r2, rsq2 = [], []
        for m2 in range(NC):
            r = sb.tile([128, S], F32R, tag="r", bufs=9, name="r2")
            nc.vector.scalar_tensor_tensor(
                out=r, in0=ps_f2[m2], scalar=W["b2"][:, m2:m2 + 1],
                in1=x1b[m2][:, :], op0=ALU.add, op1=ALU.add)
            r2.append(r)
            # eager square: lands ahead of the next st1's DVE ops in the
            # DVE FIFO, so st3b's sq-sum matmuls don't stall the PE queue
            t = sb.tile([128, S], F32R, tag="rsq3", bufs=4, name="rsq3")
            nc.vector.tensor_mul(out=t, in0=r[:, :].bitcast(F32),
                                 in1=r[:, :].bitcast(F32))
            rsq2.append(t)
        st["x1b"] = None
        st["r2"] = r2
        st["rsq2"] = rsq2

    def st3b(l, s):
        """LN2 (+ final-layer store)."""
        W = WT[l]
        st = SS[s]
        r2, rsq2 = st["r2"], st["rsq2"]
        st["r2"] = None
        st["rsq2"] = None
        if l == N_LAYERS - 1:
            xo = layernorm(r2, W["g2"], W["be2"], F32, "xOutF", 4, rsq=rsq2)
            for k in range(NC):
                nc.sync.dma_start(out=io["out"][s, 128 * k:128 * (k + 1), :],
                                  in_=xo[k][:, :])
        else:
            st["x16"] = layernorm(r2, W["g2"], W["be2"], BF16, "x16", 16,
                                  rsq=rsq2)
        if s == SQ - 1:
            WT.pop(l - 1, None)

    # Cross-layer software pipeline. Every LN stage (st2b/st3b) is emitted
    # right after a dense st1/st3a matmul stream so the PE FIFO never
    # stalls at the head on a LN DVE chain; st1 of layer l+1 fills the
    # layer boundary so the HAM clock gate stays at 8/8.
    for s in range(SQ):
        load_x0(s)
    load_layer_weights(0, part="attn")
    st1(0, 0)
    load_layer_weights(0, part="ffn")
    st1(0, 1)
    for l in range(N_LAYERS):
        last = l + 1 >= N_LAYERS
        st2(l, 0)
        st1(l, 2)
        st2(l, 1)
        st3a(l, 0)
        if not last:
            load_layer_weights(l + 1)
        st1(l, 3)
        st3b(l, 0)
        st2(l, 2)
        st3a(l, 1)
        st2(l, 3)
        st3b(l, 1)
        st3a(l, 2)
        if not last:
            st1(l + 1, 0)
        st3b(l, 2)
        st3a(l, 3)
        if not last:
            st1(l + 1, 1)
        st3b(l, 3)

    ctx.close()


def _build_program():
    _install_table_patch()
    nc = bacc.Bacc("TRN2", target_bir_lowering=False, debug=False,
                   num_devices=NCORES)
    io = {}
    io["x0T"] = nc.dram_tensor("x0T", [SQ, D, S], BF16,
                               kind="ExternalInput").ap()
    io["out"] = nc.dram_tensor("out", [SQ, D, S], F32, kind="ExternalOutput").ap()
    io["csr"] = nc.dram_tensor("csr", [SQ, H, S], F32).ap()
    for name, shape in (("wq", [N_LAYERS, D, D]), ("wk", [N_LAYERS, D, D]),
                        ("wv", [N_LAYERS, D, D]), ("wo", [N_LAYERS, D, D]),
                        ("w1", [N_LAYERS, D, F]), ("w2", [N_LAYERS, F, D])):
        io[name] = nc.dram_tensor(name, shape, BF16, kind="ExternalInput").ap()
    for name, shape in (("bq", [N_LAYERS, D]), ("bk", [N_LAYERS, D]),
                        ("bo2", [N_LAYERS, D]), ("b2", [N_LAYERS, D]),
                        ("g1", [N_LAYERS, D]), ("be1", [N_LAYERS, D]),
                        ("g2", [N_LAYERS, D]), ("be2", [N_LAYERS, D]),
                        ("b1", [N_LAYERS, F]), ("maskT", [S, SQ])):
        io[name] = nc.dram_tensor(name, shape, F32, kind="ExternalInput").ap()
    with tile.TileContext(nc) as tc:
        _emit(nc, tc, io)
    nc.compile()
    return nc


_PROGRAM = None


def _get_program():
    global _PROGRAM
    if _PROGRAM is None:
        _PROGRAM = _build_program()
    return _PROGRAM


def _positional_encoding(seq_len, d_model):
    pos = np.arange(seq_len)[:, None].astype(np.float32)
    div = np.exp(np.arange(0, d_model, 2).astype(np.float32)
                 * (-np.log(10000.0) / d_model))
    pe = np.zeros((seq_len, d_model), np.float32)
    pe[:, 0::2] = np.sin(pos * div)
    pe[:, 1::2] = np.cos(pos * div)
    return pe


def _prep_host(inputs):
    import ml_dtypes
    bf16 = ml_dtypes.bfloat16
    f = {k: np.asarray(v) for k, v in inputs.items()}
    src = f["src"].astype(np.int64)
    emb = f["emb"].astype(np.float32)
    pe = _positional_encoding(S, D)
    x0 = emb[src] * np.float32(np.sqrt(D)) + pe[None]          # [B, S, D]
    x0T = np.ascontiguousarray(x0.transpose(0, 2, 1))          # [B, D, S]
    mask = f["src_mask"].reshape(B, S)
    mask_bias = np.where(mask == 0, np.float32(-30.0),
                         np.float32(0.0)).astype(np.float32)   # [B, S]
    # fold V bias through Wo:  (v + bv) @ Wo + bo = v @ Wo + (bo + bv @ Wo)
    bo2 = f["bo"] + np.einsum("ld,lde->le", f["bv"], f["Wo"]).astype(np.float32)
    shared = {
        "wq": np.ascontiguousarray(f["Wq"][:N_LAYERS].astype(bf16)),
        "wk": np.ascontiguousarray(f["Wk"][:N_LAYERS].astype(bf16)),
        "wv": np.ascontiguousarray(f["Wv"][:N_LAYERS].astype(bf16)),
        "wo": np.ascontiguousarray(f["Wo"][:N_LAYERS].astype(bf16)),
        "w1": np.ascontiguousarray(f["W1"][:N_LAYERS].astype(bf16)),
        "w2": np.ascontiguousarray(f["W2"][:N_LAYERS].astype(bf16)),
        "bq": np.ascontiguousarray(f["bq"][:N_LAYERS]),
        "bk": np.ascontiguousarray(f["bk"][:N_LAYERS]),
        "bo2": np.ascontiguousarray(bo2[:N_LAYERS].astype(np.float32)),
        "b2": np.ascontiguousarray(f["b2"][:N_LAYERS]),
        "g1": np.ascontiguousarray(f["ln1_g"][:N_LAYERS]),
        "be1": np.ascontiguousarray(f["ln1_b"][:N_LAYERS]),
        "g2": np.ascontiguousarray(f["ln2_g"][:N_LAYERS]),
        "be2": np.ascontiguousarray(f["ln2_b"][:N_LAYERS]),
        "b1": np.ascontiguousarray(f["b1"][:N_LAYERS]),
    }
    in_maps = []
    for c in range(NCORES):
        m = dict(shared)
        m["x0T"] = np.ascontiguousarray(x0T[SQ * c:SQ * (c + 1)].astype(bf16))
        m["maskT"] = np.ascontiguousarray(
            mask_bias[SQ * c:SQ * (c + 1)].T)               # [S, SQ]
        in_maps.append(m)
    return in_maps


def run_on_device(inputs, **run_kwargs):
    """Run the model; returns (out [B,S,D] f32, BassKernelResults)."""
    nc = _get_program()
    in_maps = _prep_host(inputs)
    res = run_bass_kernel_spmd(nc, in_maps, core_ids=list(range(NCORES)),
                               **run_kwargs)
    out = np.empty((B, S, D), np.float32)
    for c in range(NCORES):
        outT = res.results[c]["out"]                         # [SQ, D, S]
        out[SQ * c:SQ * (c + 1)] = outT.transpose(0, 2, 1)
    return out, res


def kernel(**inputs) -> np.ndarray:
    out, _ = run_on_device(inputs)
    return out
